# revision 16
# baseline (speedup 1.0000x reference)
"""Causal single-head self-attention on 8 Trainium2 NeuronCores.

Problem: x:[8,2048,1024], Wq/Wk/Wv:[1024,64] ->
    out[b] = softmax(tril(x[b]Wq (x[b]Wk)^T / 64)) @ (x[b]Wv)   [8,2048,64]

Sharding: data-parallel over batch -- core b gets batch element b.
Weights replicated.

Per-core algorithm (fp32 matmuls on TRN2 lower to 2x LOW_HIGH PE passes,
so all matmul operands are bf16 with fp32 PSUM accumulation; measured
end-to-end rel err ~3.8e-3 against the fp32 reference):
  - host pre-swizzles x[b] into the exact SBUF layout xp[128, 8, 2048]
    (partition, e-chunk, seq) in bf16 so every DMA is dense, and
    likewise the weights; kernel output is out^T [64, S] fp32,
    un-transposed on the host
  - per q-block of 512 (pipelined with the xT DMA):
      qkT[128, qb]: rows 0:64 = q^T, 64:128 = k^T via packed projection
      (lhsT=[Wq|Wk][e]); kT DMA-shifted to partitions 0:64 and qT
      DMA-shifted up to partitions 64:128 so score matmuls can be
      row-packed two-at-a-time on PE row groups 0:64 / 64:128;
      v^T projected likewise then PE-transposed to natural v[s,64] bf16
      with a ones column appended -> v_aug[s, 65]
      attention: scores^T[kc, qb] = kT_kc.T @ qT_qb (K=64, fp32 psum);
      exp via ACT (scale=1/64, fp32 in, bf16 out); causal = skip
      above-diagonal chunks + truncate diagonal chunks' q-range +
      gpsimd affine_select triangular mask; out^T psum[65, qb] +=
      v_aug[kc].T @ expT; row 64 accumulates the softmax denominators
      normalize: reciprocal of row 64 -> broadcast over partitions 0:64
      via a ones[64,64] matmul -> elementwise multiply -> out^T store
"""

import os
from contextlib import ExitStack

import numpy as np

import concourse.bass as bass
import concourse.mybir as mybir
import concourse.tile as tile
from concourse import bacc
from concourse.bass_utils import run_bass_kernel_spmd
from concourse.masks import make_identity

B, S, E, H = 8, 2048, 1024, 64
P = 128
QB = 512  # q-block (psum free dim)
F32 = mybir.dt.float32
BF16 = mybir.dt.bfloat16


def build_kernel_body(tc, xp_d, wqk_d, wv_d, out_d, s=S, e_dim=E):
    nc = tc.nc
    EC = e_dim // P  # e-chunks
    NQB = s // QB    # q-blocks
    NT = s // P      # s-tiles of 128
    KPQ = QB // P    # k-chunks per q-block (4)

    ctx = ExitStack()
    with ctx:
        const = ctx.enter_context(tc.tile_pool(name="const", bufs=1))
        big = ctx.enter_context(tc.tile_pool(name="big", bufs=1))

        # weights on the ACT HWDGE ring so the Sync ring starts on x
        wqk_sb = const.tile([P, EC, 2 * H], BF16)
        nc.scalar.dma_start(wqk_sb[:], wqk_d[:])
        wv_sb = const.tile([P, EC, H], BF16)
        nc.scalar.dma_start(wv_sb[:], wv_d[:])
        ident_bf = const.tile([P, P], BF16)
        make_identity(nc, ident_bf[:])
        ones_sb = const.tile([H, H], F32)
        nc.gpsimd.memset(ones_sb[:], 1.0)
        recb_sb = const.tile([H, QB], F32)
        nc.gpsimd.memset(recb_sb[:], 0.0)  # rows 1:64 stay zero

        # x blocks q-block-major; first q-block split so the first
        # projection matmul can start after 256 KB
        xp_sb = big.tile([P, EC, s], BF16)
        for g in range(EC // 2):
            nc.sync.dma_start(
                xp_sb[:, 2 * g:2 * g + 2, 0:QB],
                xp_d[:, 2 * g:2 * g + 2, 0:QB])
        for qb in range(1, NQB):
            nc.sync.dma_start(
                xp_sb[:, :, qb * QB:(qb + 1) * QB],
                xp_d[:, :, qb * QB:(qb + 1) * QB])

        qkT_sb = big.tile([P, s], BF16)  # rows 0:64 qT, rows 64:128 kT
        kT_sb = big.tile([H, s], BF16)   # kT at base partition 0
        qT2_sb = big.tile([P, s], BF16)  # qT duplicated at rows 64:128
        vT_sb = big.tile([H, s], BF16)
        v_sb = big.tile([P, NT, H + 1], BF16)  # natural v + ones col
        outT_sb = big.tile([H, s], F32)

        nc.gpsimd.memset(v_sb[:, :, H:H + 1], 1.0)

        # PSUM budget (8 banks): pqk 1 + pvt 1 + ps 3 + po 1 + pb 1
        # + ptr 1 = 8
        pqk = ctx.enter_context(tc.tile_pool(name="ps_qk", bufs=1, space="PSUM"))
        pvt = ctx.enter_context(tc.tile_pool(name="ps_vt", bufs=1, space="PSUM"))
        ps = ctx.enter_context(tc.tile_pool(name="ps_s", bufs=3, space="PSUM"))
        po = ctx.enter_context(tc.tile_pool(name="ps_o", bufs=1, space="PSUM"))
        pb = ctx.enter_context(tc.tile_pool(name="ps_b", bufs=1, space="PSUM"))
        ptr = ctx.enter_context(tc.tile_pool(name="ps_tr", bufs=1, space="PSUM"))
        ep = ctx.enter_context(tc.tile_pool(name="expp", bufs=6))
        sp = ctx.enter_context(tc.tile_pool(name="smalls", bufs=4))

        psum_o_pend = [None] * NQB

        def normalize(qb):
            # psum_o[65, QB]: rows 0:64 = unnormalized out^T, row 64 =
            # softmax denominators. recip -> broadcast over partitions
            # (ones[64,64] @ [recip; zeros]) -> multiply -> store out^T.
            qsl = slice(qb * QB, (qb + 1) * QB)
            psum_o = psum_o_pend[qb]
            nc.vector.reciprocal(recb_sb[0:1, :], psum_o[H:H + 1, :])
            psum_b = pb.tile([H, QB], F32)
            nc.tensor.matmul(
                psum_b[:], lhsT=ones_sb[:], rhs=recb_sb[:],
                start=True, stop=True)
            bcast = sp.tile([H, QB], F32, tag="bc")
            nc.vector.tensor_copy(bcast[:], psum_b[:])
            nc.vector.tensor_mul(
                out=outT_sb[:, qsl], in0=psum_o[0:H, :], in1=bcast[:])
            nc.sync.dma_start(out_d[:, qsl], outT_sb[:, qsl])

        for qb in range(NQB):
            qsl = slice(qb * QB, (qb + 1) * QB)
            # ---- projections for this q-block ----
            psum_qk = pqk.tile([P, QB], F32, tag="qk")
            psum_vT = pvt.tile([H, QB], F32, tag="vt")
            for ec in range(EC):
                nc.tensor.matmul(
                    psum_qk[:], lhsT=wqk_sb[:, ec, :],
                    rhs=xp_sb[:, ec, qsl],
                    start=(ec == 0), stop=(ec == EC - 1))
                nc.tensor.matmul(
                    psum_vT[:], lhsT=wv_sb[:, ec, :],
                    rhs=xp_sb[:, ec, qsl],
                    start=(ec == 0), stop=(ec == EC - 1))
            nc.vector.tensor_copy(qkT_sb[:, qsl], psum_qk[:])
            nc.gpsimd.dma_start(kT_sb[:, qsl], qkT_sb[H:P, qsl])
            nc.gpsimd.dma_start(qT2_sb[H:P, qsl], qkT_sb[0:H, qsl])
            nc.vector.tensor_copy(vT_sb[:, qsl], psum_vT[:])
            for t in range(qb * KPQ, (qb + 1) * KPQ):
                pvtr = ptr.tile([P, H], BF16, tag="tr")
                nc.tensor.transpose(
                    pvtr[:], vT_sb[:, t * P:(t + 1) * P], ident_bf[0:H, 0:H])
                nc.vector.tensor_copy(v_sb[:, t, 0:H], pvtr[:])

            # previous q-block's normalize overlaps this one's attention
            if qb > 0:
                normalize(qb - 1)

            # ---- attention for this q-block ----
            nkc = (qb + 1) * KPQ
            psum_o = po.tile([H + 1, QB], F32)
            psum_o_pend[qb] = psum_o
            for pr in range(nkc // 2):
                kc0, kc1 = 2 * pr, 2 * pr + 1
                # row-packed pair: kc0 on PE rows 0:64, kc1 on rows
                # 64:128 (kT lives at rows 64:128 of qkT_sb; qT
                # duplicated there); the two matmuls run concurrently
                psum_prs = []
                offs = []
                for i, kc in enumerate((kc0, kc1)):
                    o = max(0, kc * P - qb * QB)
                    offs.append(o)
                    psum_s = ps.tile([P, QB], F32, tag="sc")
                    psum_prs.append(psum_s)
                    if i == 0:
                        nc.tensor.matmul(
                            psum_s[:, o:],
                            lhsT=kT_sb[:, kc * P:(kc + 1) * P],
                            rhs=qkT_sb[0:H, qsl][:, o:],
                            start=True, stop=True)
                    else:
                        nc.tensor.matmul(
                            psum_s[:, o:],
                            lhsT=qkT_sb[H:P, kc * P:(kc + 1) * P],
                            rhs=qT2_sb[H:P, qsl][:, o:],
                            start=True, stop=True)
                for i, (kc, o) in enumerate(((kc0, offs[0]), (kc1, offs[1]))):
                    et = ep.tile([P, QB], BF16)
                    nc.scalar.activation(
                        et[:, o:], psum_prs[i][:, o:],
                        mybir.ActivationFunctionType.Exp, scale=1.0 / H)
                    if kc * P - qb * QB >= 0:
                        # diagonal chunk: keep where q >= k (j - p >= 0)
                        nc.gpsimd.affine_select(
                            out=et[:, o:], in_=et[:, o:],
                            compare_op=mybir.AluOpType.is_ge,
                            fill=0.0, base=0,
                            channel_multiplier=-1,
                            pattern=[[1, QB - o]])
                    nc.tensor.matmul(
                        psum_o[:, o:],
                        lhsT=v_sb[:, kc, :],
                        rhs=et[:, o:],
                        start=(kc == 0), stop=(kc == nkc - 1))

        normalize(NQB - 1)


def build_bass(s=S, e_dim=E, n_cores=B):
    nc = bacc.Bacc(
        "TRN2", target_bir_lowering=False, debug=False, num_devices=n_cores)
    EC = e_dim // P
    xp_d = nc.dram_tensor("xp", [P, EC, s], BF16, kind="ExternalInput").ap()
    wqk_d = nc.dram_tensor(
        "wqk", [P, EC, 2 * H], BF16, kind="ExternalInput").ap()
    wv_d = nc.dram_tensor("wv", [P, EC, H], BF16, kind="ExternalInput").ap()
    out_d = nc.dram_tensor("out", [H, s], F32, kind="ExternalOutput").ap()
    with tile.TileContext(nc) as tc:
        build_kernel_body(tc, xp_d, wqk_d, wv_d, out_d, s=s, e_dim=e_dim)
    nc.compile()
    return nc


_nc_cache = None


def _ensure_ntff_hook():
    """Dev-only: provide the antenv.axon_hooks shim so trace=True can
    capture NTFF profiles through libaxon_pjrt.so in this container."""
    import sys
    import types
    import ctypes
    import contextlib

    try:
        from antenv.axon_hooks import get_axon_ntff_profile_hook  # noqa
        return
    except ImportError:
        pass
    import antenv

    mod = types.ModuleType("antenv.axon_hooks")
    _h = [None]
    mod.set_axon_ntff_profile_hook = lambda h: _h.__setitem__(0, h)
    mod.get_axon_ntff_profile_hook = lambda: _h[0]
    sys.modules["antenv.axon_hooks"] = mod
    antenv.axon_hooks = mod

    so_path = "/opt/axon/libaxon_pjrt.so"
    lib = ctypes.CDLL(so_path)
    if not hasattr(lib, "axon_start_nrt_profile"):
        return
    lib.axon_start_nrt_profile.argtypes = [
        ctypes.POINTER(ctypes.c_int64), ctypes.c_size_t]
    lib.axon_start_nrt_profile.restype = ctypes.c_int64
    lib.axon_stop_nrt_profile.argtypes = [ctypes.c_char_p]
    lib.axon_stop_nrt_profile.restype = ctypes.c_int64

    @contextlib.contextmanager
    def _hook(output_dir, device_ids):
        import jax
        jax.devices()
        if device_ids:
            ids = (ctypes.c_int64 * len(device_ids))(*device_ids)
            rc = lib.axon_start_nrt_profile(ids, len(device_ids))
        else:
            rc = lib.axon_start_nrt_profile(None, 0)
        if rc != 0:
            raise RuntimeError(f"axon_start_nrt_profile rc={rc}")
        try:
            yield
        finally:
            n = lib.axon_stop_nrt_profile(str(output_dir).encode())
            print(f"profile: {n} file(s) written to {output_dir}")

    mod.set_axon_ntff_profile_hook(_hook)

    # no bucket access in this container; keep artifacts local
    import concourse.bass_utils as bu
    bu.upload_artifacts = lambda tmpdir: tmpdir


def _swizzle(a, ec, p):
    """[E, M] -> [P, EC, M] with [pp, c, m] = a[c*p + pp, m]."""
    return np.ascontiguousarray(a.reshape(ec, p, a.shape[-1]).transpose(1, 0, 2))


def kernel(x, Wq, Wk, Wv):
    global _nc_cache
    import ml_dtypes
    bf = ml_dtypes.bfloat16

    x = np.asarray(x, dtype=np.float32)
    Wq = np.asarray(Wq, dtype=np.float32)
    Wk = np.asarray(Wk, dtype=np.float32)
    Wv = np.asarray(Wv, dtype=np.float32)

    if _nc_cache is None:
        _nc_cache = build_bass()
    nc = _nc_cache

    EC = E // P
    wqk = _swizzle(np.concatenate([Wq, Wk], axis=1).astype(bf), EC, P)
    wv = _swizzle(Wv.astype(bf), EC, P)
    in_maps = []
    for b in range(B):
        in_maps.append({
            "xp": _swizzle(x[b].T.astype(bf), EC, P),
            "wqk": wqk,
            "wv": wv,
        })

    trace = bool(int(os.environ.get("ATTN_TRACE", "0")))
    if trace:
        _ensure_ntff_hook()
    res = run_bass_kernel_spmd(
        nc, in_maps, core_ids=list(range(B)), trace=trace)
    if trace and res.exec_time_ns is not None:
        print(f"HW exec time: {res.exec_time_ns} ns")
        kernel.last_exec_time_ns = res.exec_time_ns
        kernel.last_results = res
    # out^T [64, S] per core -> [B, S, 64]
    out = np.stack(
        [np.ascontiguousarray(res.results[b]["out"].T) for b in range(B)],
        axis=0)
    return out


# revision 19
# speedup vs baseline: 1.2100x; 1.2100x over previous
"""Causal single-head self-attention on 8 Trainium2 NeuronCores.

Problem: x:[8,2048,1024], Wq/Wk/Wv:[1024,64] ->
    out[b] = softmax(tril(x[b]Wq (x[b]Wk)^T / 64)) @ (x[b]Wv)   [8,2048,64]

Sharding: data-parallel over batch -- core b gets batch element b.
Weights replicated.

Per-core algorithm (fp32 matmuls on TRN2 lower to 2x LOW_HIGH PE passes,
so all matmul operands are bf16 with fp32 PSUM accumulation; measured
end-to-end rel err ~3.8e-3 against the fp32 reference):
  - host pre-swizzles x[b] into the exact SBUF layout xp[128, 8, 2048]
    (partition, e-chunk, seq) in bf16 so every DMA is dense, and
    likewise the weights; kernel output is out^T [64, S] fp32,
    un-transposed on the host
  - per q-block of 512 (pipelined with the xT DMA):
      qkT[128, qb]: rows 0:64 = q^T, 64:128 = k^T via packed projection
      (lhsT=[Wq|Wk][e]); kT DMA-shifted to partitions 0:64 and qT
      DMA-shifted up to partitions 64:128 so score matmuls can be
      row-packed two-at-a-time on PE row groups 0:64 / 64:128;
      v^T projected likewise then PE-transposed to natural v[s,64] bf16
      with a ones column appended -> v_aug[s, 65]
      attention: scores^T[kc, qb] = kT_kc.T @ qT_qb (K=64, fp32 psum);
      exp via ACT (scale=1/64, fp32 in, bf16 out); causal = skip
      above-diagonal chunks + truncate diagonal chunks' q-range +
      gpsimd affine_select triangular mask; out^T psum[65, qb] +=
      v_aug[kc].T @ expT; row 64 accumulates the softmax denominators
      normalize: reciprocal of row 64 -> broadcast over partitions 0:64
      via a ones[64,64] matmul -> elementwise multiply -> out^T store
"""

import os
from contextlib import ExitStack

import numpy as np

import concourse.bass as bass
import concourse.mybir as mybir
import concourse.tile as tile
from concourse import bacc
from concourse.bass_utils import run_bass_kernel_spmd
from concourse.masks import make_identity

B, S, E, H = 8, 2048, 1024, 64
P = 128
QB = 512  # q-block (psum free dim)
F32 = mybir.dt.float32
BF16 = mybir.dt.bfloat16


def build_kernel_body(tc, xp_d, wqk_d, wv_d, out_d, s=S, e_dim=E):
    nc = tc.nc
    EC = e_dim // P  # e-chunks
    NQB = s // QB    # q-blocks
    NT = s // P      # s-tiles of 128
    KPQ = QB // P    # k-chunks per q-block (4)

    ctx = ExitStack()
    with ctx:
        const = ctx.enter_context(tc.tile_pool(name="const", bufs=1))
        big = ctx.enter_context(tc.tile_pool(name="big", bufs=1))

        # weights on the ACT HWDGE ring so the Sync ring starts on x
        wqk_sb = const.tile([P, EC, 2 * H], BF16)
        nc.scalar.dma_start(wqk_sb[:], wqk_d[:])
        wv_sb = const.tile([P, EC, H], BF16)
        nc.scalar.dma_start(wv_sb[:], wv_d[:])
        ident_bf = const.tile([P, P], BF16)
        make_identity(nc, ident_bf[:])
        ones_sb = const.tile([H, H], F32)
        nc.gpsimd.memset(ones_sb[:], 1.0)
        recb_sb = const.tile([H, QB], F32)
        nc.gpsimd.memset(recb_sb[:], 0.0)  # rows 1:64 stay zero

        # x blocks q-block-major; first q-block split so the first
        # projection matmul can start after 256 KB
        xp_sb = big.tile([P, EC, s], BF16)
        for qb in range(NQB):
            for g in range(EC // 2):
                nc.sync.dma_start(
                    xp_sb[:, 2 * g:2 * g + 2, qb * QB:(qb + 1) * QB],
                    xp_d[:, 2 * g:2 * g + 2, qb * QB:(qb + 1) * QB])

        qkT_sb = big.tile([P, s], BF16)  # rows 0:64 qT, rows 64:128 kT
        kT_sb = big.tile([H, s], BF16)   # kT at base partition 0
        qT2_sb = big.tile([P, s], BF16)  # qT duplicated at rows 64:128
        vT_sb = big.tile([H, s], BF16)
        v_sb = big.tile([P, NT, H + 1], BF16)  # natural v + ones col
        outT_sb = big.tile([H, s], F32)

        nc.gpsimd.memset(v_sb[:, :, H:H + 1], 1.0)

        # PSUM budget (8 banks): pqk 1 + pvt 1 + ps 3 (also serves the
        # bcast matmul) + po 2 + ptr 1 = 8
        pqk = ctx.enter_context(tc.tile_pool(name="ps_qk", bufs=1, space="PSUM"))
        pvt = ctx.enter_context(tc.tile_pool(name="ps_vt", bufs=1, space="PSUM"))
        ps = ctx.enter_context(tc.tile_pool(name="ps_s", bufs=3, space="PSUM"))
        po = ctx.enter_context(tc.tile_pool(name="ps_o", bufs=2, space="PSUM"))
        ptr = ctx.enter_context(tc.tile_pool(name="ps_tr", bufs=1, space="PSUM"))
        ep = ctx.enter_context(tc.tile_pool(name="expp", bufs=6))
        sp = ctx.enter_context(tc.tile_pool(name="smalls", bufs=4))

        psum_o_pend = [None] * NQB

        def normalize(qb):
            # psum_o[65, QB]: rows 0:64 = unnormalized out^T, row 64 =
            # softmax denominators. 1/den = exp(-ln(den)) on ACT (the
            # DVE reciprocal streams the free dim at ~6.5 cyc/elem on a
            # single lane -- 3.4 us for 512), then broadcast over
            # partitions via ones[64,64] @ [recip; zeros], multiply,
            # store out^T.
            qsl = slice(qb * QB, (qb + 1) * QB)
            psum_o = psum_o_pend[qb]
            lnd = sp.tile([1, QB], F32, tag="lnd")
            nc.scalar.activation(
                lnd[:], psum_o[H:H + 1, :], mybir.ActivationFunctionType.Ln)
            nc.scalar.activation(
                recb_sb[0:1, :], lnd[:],
                mybir.ActivationFunctionType.Exp, scale=-1.0)
            psum_b = ps.tile([P, QB], F32, tag="sc")
            nc.tensor.matmul(
                psum_b[0:H, :], lhsT=ones_sb[:], rhs=recb_sb[:],
                start=True, stop=True)
            bcast = sp.tile([H, QB], F32, tag="bc")
            nc.vector.tensor_copy(bcast[:], psum_b[0:H, :])
            nc.vector.tensor_mul(
                out=outT_sb[:, qsl], in0=psum_o[0:H, :], in1=bcast[:])
            nc.sync.dma_start(out_d[:, qsl], outT_sb[:, qsl])

        for qb in range(NQB):
            qsl = slice(qb * QB, (qb + 1) * QB)
            # ---- projections for this q-block ----
            psum_qk = pqk.tile([P, QB], F32, tag="qk")
            psum_vT = pvt.tile([H, QB], F32, tag="vt")
            for ec in range(EC):
                nc.tensor.matmul(
                    psum_qk[:], lhsT=wqk_sb[:, ec, :],
                    rhs=xp_sb[:, ec, qsl],
                    start=(ec == 0), stop=(ec == EC - 1))
                nc.tensor.matmul(
                    psum_vT[:], lhsT=wv_sb[:, ec, :],
                    rhs=xp_sb[:, ec, qsl],
                    start=(ec == 0), stop=(ec == EC - 1))
            nc.vector.tensor_copy(qkT_sb[:, qsl], psum_qk[:])
            nc.gpsimd.dma_start(kT_sb[:, qsl], qkT_sb[H:P, qsl])
            nc.gpsimd.dma_start(qT2_sb[H:P, qsl], qkT_sb[0:H, qsl])
            nc.vector.tensor_copy(vT_sb[:, qsl], psum_vT[:])
            for t in range(qb * KPQ, (qb + 1) * KPQ):
                pvtr = ptr.tile([P, H], BF16, tag="tr")
                nc.tensor.transpose(
                    pvtr[:], vT_sb[:, t * P:(t + 1) * P], ident_bf[0:H, 0:H])
                nc.vector.tensor_copy(v_sb[:, t, 0:H], pvtr[:])

            # previous q-block's normalize overlaps this one's attention
            if qb > 0:
                normalize(qb - 1)

            # ---- attention for this q-block ----
            nkc = (qb + 1) * KPQ
            psum_o = po.tile([H + 1, QB], F32)
            psum_o_pend[qb] = psum_o
            for pr in range(nkc // 2):
                kc0, kc1 = 2 * pr, 2 * pr + 1
                # row-packed pair: kc0 on PE rows 0:64, kc1 on rows
                # 64:128 (kT lives at rows 64:128 of qkT_sb; qT
                # duplicated there); the two matmuls run concurrently
                psum_prs = []
                offs = []
                for i, kc in enumerate((kc0, kc1)):
                    o = max(0, kc * P - qb * QB)
                    offs.append(o)
                    psum_s = ps.tile([P, QB], F32, tag="sc")
                    psum_prs.append(psum_s)
                    if i == 0:
                        nc.tensor.matmul(
                            psum_s[:, o:],
                            lhsT=kT_sb[:, kc * P:(kc + 1) * P],
                            rhs=qkT_sb[0:H, qsl][:, o:],
                            start=True, stop=True)
                    else:
                        nc.tensor.matmul(
                            psum_s[:, o:],
                            lhsT=qkT_sb[H:P, kc * P:(kc + 1) * P],
                            rhs=qT2_sb[H:P, qsl][:, o:],
                            start=True, stop=True)
                for i, (kc, o) in enumerate(((kc0, offs[0]), (kc1, offs[1]))):
                    et = ep.tile([P, QB], BF16)
                    nc.scalar.activation(
                        et[:, o:], psum_prs[i][:, o:],
                        mybir.ActivationFunctionType.Exp, scale=1.0 / H)
                    if kc * P - qb * QB >= 0:
                        # diagonal chunk: keep where q >= k (j - p >= 0)
                        nc.gpsimd.affine_select(
                            out=et[:, o:], in_=et[:, o:],
                            compare_op=mybir.AluOpType.is_ge,
                            fill=0.0, base=0,
                            channel_multiplier=-1,
                            pattern=[[1, QB - o]])
                    nc.tensor.matmul(
                        psum_o[:, o:],
                        lhsT=v_sb[:, kc, :],
                        rhs=et[:, o:],
                        start=(kc == 0), stop=(kc == nkc - 1))

        normalize(NQB - 1)


def build_bass(s=S, e_dim=E, n_cores=B):
    nc = bacc.Bacc(
        "TRN2", target_bir_lowering=False, debug=False, num_devices=n_cores)
    EC = e_dim // P
    xp_d = nc.dram_tensor("xp", [P, EC, s], BF16, kind="ExternalInput").ap()
    wqk_d = nc.dram_tensor(
        "wqk", [P, EC, 2 * H], BF16, kind="ExternalInput").ap()
    wv_d = nc.dram_tensor("wv", [P, EC, H], BF16, kind="ExternalInput").ap()
    out_d = nc.dram_tensor("out", [H, s], F32, kind="ExternalOutput").ap()
    with tile.TileContext(nc) as tc:
        build_kernel_body(tc, xp_d, wqk_d, wv_d, out_d, s=s, e_dim=e_dim)
    nc.compile()
    return nc


_nc_cache = None


def _ensure_ntff_hook():
    """Dev-only: provide the antenv.axon_hooks shim so trace=True can
    capture NTFF profiles through libaxon_pjrt.so in this container."""
    import sys
    import types
    import ctypes
    import contextlib

    try:
        from antenv.axon_hooks import get_axon_ntff_profile_hook  # noqa
        return
    except ImportError:
        pass
    import antenv

    mod = types.ModuleType("antenv.axon_hooks")
    _h = [None]
    mod.set_axon_ntff_profile_hook = lambda h: _h.__setitem__(0, h)
    mod.get_axon_ntff_profile_hook = lambda: _h[0]
    sys.modules["antenv.axon_hooks"] = mod
    antenv.axon_hooks = mod

    so_path = "/opt/axon/libaxon_pjrt.so"
    lib = ctypes.CDLL(so_path)
    if not hasattr(lib, "axon_start_nrt_profile"):
        return
    lib.axon_start_nrt_profile.argtypes = [
        ctypes.POINTER(ctypes.c_int64), ctypes.c_size_t]
    lib.axon_start_nrt_profile.restype = ctypes.c_int64
    lib.axon_stop_nrt_profile.argtypes = [ctypes.c_char_p]
    lib.axon_stop_nrt_profile.restype = ctypes.c_int64

    @contextlib.contextmanager
    def _hook(output_dir, device_ids):
        import jax
        jax.devices()
        if device_ids:
            ids = (ctypes.c_int64 * len(device_ids))(*device_ids)
            rc = lib.axon_start_nrt_profile(ids, len(device_ids))
        else:
            rc = lib.axon_start_nrt_profile(None, 0)
        if rc != 0:
            raise RuntimeError(f"axon_start_nrt_profile rc={rc}")
        try:
            yield
        finally:
            n = lib.axon_stop_nrt_profile(str(output_dir).encode())
            print(f"profile: {n} file(s) written to {output_dir}")

    mod.set_axon_ntff_profile_hook(_hook)

    # no bucket access in this container; keep artifacts local
    import concourse.bass_utils as bu
    bu.upload_artifacts = lambda tmpdir: tmpdir


def _swizzle(a, ec, p):
    """[E, M] -> [P, EC, M] with [pp, c, m] = a[c*p + pp, m]."""
    return np.ascontiguousarray(a.reshape(ec, p, a.shape[-1]).transpose(1, 0, 2))


def kernel(x, Wq, Wk, Wv):
    global _nc_cache
    import ml_dtypes
    bf = ml_dtypes.bfloat16

    x = np.asarray(x, dtype=np.float32)
    Wq = np.asarray(Wq, dtype=np.float32)
    Wk = np.asarray(Wk, dtype=np.float32)
    Wv = np.asarray(Wv, dtype=np.float32)

    if _nc_cache is None:
        _nc_cache = build_bass()
    nc = _nc_cache

    EC = E // P
    wqk = _swizzle(np.concatenate([Wq, Wk], axis=1).astype(bf), EC, P)
    wv = _swizzle(Wv.astype(bf), EC, P)
    in_maps = []
    for b in range(B):
        in_maps.append({
            "xp": _swizzle(x[b].T.astype(bf), EC, P),
            "wqk": wqk,
            "wv": wv,
        })

    trace = bool(int(os.environ.get("ATTN_TRACE", "0")))
    if trace:
        _ensure_ntff_hook()
    res = run_bass_kernel_spmd(
        nc, in_maps, core_ids=list(range(B)), trace=trace)
    if trace and res.exec_time_ns is not None:
        print(f"HW exec time: {res.exec_time_ns} ns")
        kernel.last_exec_time_ns = res.exec_time_ns
        kernel.last_results = res
    # out^T [64, S] per core -> [B, S, 64]
    out = np.stack(
        [np.ascontiguousarray(res.results[b]["out"].T) for b in range(B)],
        axis=0)
    return out
